# revision 10
# baseline (speedup 1.0000x reference)
"""3-layer GAT (DGL GATConv style) on 8 Trainium2 NeuronCores.

Sharding: nodes are padded to NPAD and partitioned into 8 contiguous,
tile-aligned ranges (one per core).  Each core owns the destination
segments of its node range and processes the in-edges of those nodes.
Per layer:
  1. table pass: each core computes ft/el/er for its own shard from the
     transposed hidden state (hT) and small replicated weights,
  2. AllGather of the ft table (bf16) and el vector across cores,
  3. edge pass: per-edge gathers of ft[src] / el[src] / er[dst] via
     dma_gather, edge softmax (max-free, mathematically identical), and
     segment-sum aggregation via one-hot matmuls into PSUM.

Edges are bucketed by (dst tile, src chunk) on the host; each bucket is
padded to a fixed number of 128-edge groups so that the SPMD program is
identical across cores (only input data differs).
"""

import numpy as np
import ml_dtypes

import concourse.bass as bass
import concourse.bacc as bacc
import concourse.mybir as mybir
import concourse.tile as tile
from concourse.bass_utils import run_bass_kernel_spmd
from concourse.masks import make_identity

F32 = mybir.dt.float32
BF16 = mybir.dt.bfloat16
I16 = mybir.dt.int16

P = 128


class Cfg:
    """Geometry of the sharded GAT kernel."""

    def __init__(self, n_real, e_real, tiles_per_core=98, tb=7, ncores=8,
                 fin=128, heads=4, dh=32, out=64, neg=0.2):
        self.N = n_real
        self.E = e_real
        self.NCORES = ncores
        self.TILES = tiles_per_core          # node tiles per core
        self.SHARD = tiles_per_core * P      # nodes per core
        self.NPAD = ncores * self.SHARD      # padded node count
        self.CHUNKS = 4
        assert self.NPAD % self.CHUNKS == 0
        self.CR = self.NPAD // self.CHUNKS   # rows per src chunk
        assert self.CR <= 32768 and self.SHARD <= 32768  # int16 gather idx
        self.TB = tb                         # tiles per batch
        assert tiles_per_core % tb == 0
        self.NB = tiles_per_core // tb       # batches
        self.FIN = fin
        self.HEADS = heads
        self.DH = dh
        self.OUT = out
        self.NEG = neg
        assert fin == heads * dh == P
        self.G = None                        # slot groups per (tile, chunk); data-derived


# ----------------------------------------------------------------------------
# Host-side preprocessing
# ----------------------------------------------------------------------------

def _wrap_idx(vals, nidx):
    """int array [nidx] -> [128, nidx//16] int16, idx i at [i%16, i//16],
    replicated into all 8 16-partition bands (one per GPSIMD Q7 core)."""
    band = np.asarray(vals, dtype=np.int16).reshape(-1, 16).T
    return np.tile(band, (8, 1))


def prep_edges(cfg, src, dst):
    """Bucket edges by (core, batch, chunk, tile-in-batch); pad buckets to
    a uniform G*128 slots.  Returns per-core idx/dstl arrays + G."""
    c = cfg
    src = np.asarray(src, dtype=np.int64)
    dst = np.asarray(dst, dtype=np.int64)

    core = dst // c.SHARD
    tloc = (dst % c.SHARD) // P           # tile within core
    b = tloc // c.TB                      # batch
    ti = tloc % c.TB                      # tile within batch
    chunk = src // c.CR

    nbuck = c.NCORES * c.NB * c.CHUNKS * c.TB
    bucket = ((core * c.NB + b) * c.CHUNKS + chunk) * c.TB + ti
    counts = np.bincount(bucket, minlength=nbuck)
    gmax = int(counts.max())
    G = (gmax + P - 1) // P
    c.G = G
    SLOT = c.TB * G                       # 128-edge groups per (batch, chunk)
    NIDX = SLOT * P                       # gather indices per (batch, chunk)
    cap = G * P

    order = np.argsort(bucket, kind="stable")
    sorted_bucket = bucket[order]
    # position of each edge within its bucket
    boundaries = np.concatenate([[0], np.cumsum(counts)])
    within = np.arange(len(src)) - boundaries[sorted_bucket]

    # flat slot index per (core): layout [NB, CHUNKS, TB, cap]
    sb = sorted_bucket
    s_core = sb // (c.NB * c.CHUNKS * c.TB)
    rem = sb % (c.NB * c.CHUNKS * c.TB)
    s_b = rem // (c.CHUNKS * c.TB)
    rem = rem % (c.CHUNKS * c.TB)
    s_chunk = rem // c.TB
    s_ti = rem % c.TB
    flat = ((s_b * c.CHUNKS + s_chunk) * c.TB + s_ti) * cap + within

    src_s = src[order]
    dst_s = dst[order]

    per_core_ft = np.zeros((c.NCORES, c.NB * c.CHUNKS * c.TB * cap), np.int16)
    per_core_er = np.zeros_like(per_core_ft)
    per_core_dl = np.full((c.NCORES, c.NB * c.CHUNKS * c.TB * cap), -1.0, np.float32)
    for k in range(c.NCORES):
        m = s_core == k
        per_core_ft[k, flat[m]] = (src_s[m] % c.CR).astype(np.int16)
        per_core_er[k, flat[m]] = (dst_s[m] % c.SHARD).astype(np.int16)
        per_core_dl[k, flat[m]] = (dst_s[m] % P).astype(np.float32)

    # reshape to device input layouts
    ftidx = np.zeros((c.NCORES, c.NB * c.CHUNKS, P, NIDX // 16), np.int16)
    erridx = np.zeros_like(ftidx)
    dstl = np.zeros((c.NCORES, c.NB * c.CHUNKS, P, SLOT), np.float32)
    v_ft = per_core_ft.reshape(c.NCORES, c.NB, c.CHUNKS, NIDX)
    v_er = per_core_er.reshape(c.NCORES, c.NB, c.CHUNKS, NIDX)
    v_dl = per_core_dl.reshape(c.NCORES, c.NB, c.CHUNKS, SLOT, P)
    for k in range(c.NCORES):
        for bb in range(c.NB):
            for ch in range(c.CHUNKS):
                bc = bb * c.CHUNKS + ch
                ftidx[k, bc] = _wrap_idx(v_ft[k, bb, ch], NIDX)
                erridx[k, bc] = _wrap_idx(v_er[k, bb, ch], NIDX)
                dstl[k, bc] = v_dl[k, bb, ch].T  # [P, SLOT]
    return ftidx, erridx, dstl


def prep_weights(cfg, W, al, ar):
    """[W | Wl | Wr] with Wl[f,h] = sum_d W[f, h*D+d]*al[h,d]."""
    H, D = al.shape
    Wv = W.reshape(W.shape[0], H, D)
    Wl = np.einsum("fhd,hd->fh", Wv, al)
    Wr = np.einsum("fhd,hd->fh", Wv, ar)
    return np.concatenate([W, Wl, Wr], axis=1).astype(np.float32)


# ----------------------------------------------------------------------------
# Device program
# ----------------------------------------------------------------------------

def build_program(cfg, bench_compute=0, bench_ag=0):
    c = cfg
    G = c.G
    SLOT = c.TB * G
    NIDX = SLOT * P
    H0, D0, F0 = c.HEADS, c.DH, P     # layers 0/1
    H2, D2, F2 = 1, c.OUT, c.OUT      # layer 2

    nc = bacc.Bacc("TRN2", target_bir_lowering=False, debug=False,
                   num_devices=c.NCORES, num_swdge_queues=4)

    # ---- I/O ----
    featT_own = nc.dram_tensor("featT_own", [P, c.SHARD], F32, kind="ExternalInput")
    ftidx_in = nc.dram_tensor("ftidx", [c.NB * c.CHUNKS, P, NIDX // 16], I16,
                              kind="ExternalInput")
    erridx_in = nc.dram_tensor("erridx", [c.NB * c.CHUNKS, P, NIDX // 16], I16,
                               kind="ExternalInput")
    dstl_in = nc.dram_tensor("dstl", [c.NB * c.CHUNKS, P, SLOT], F32,
                             kind="ExternalInput")
    wc0_in = nc.dram_tensor("wc0", [P, F0 + 2 * H0], F32, kind="ExternalInput")
    wc1_in = nc.dram_tensor("wc1", [P, F0 + 2 * H0], F32, kind="ExternalInput")
    wc2_in = nc.dram_tensor("wc2", [P, F2 + 2 * H2], F32, kind="ExternalInput")
    bias0_in = nc.dram_tensor("bias0", [P, F0], F32, kind="ExternalInput")
    bias1_in = nc.dram_tensor("bias1", [P, F0], F32, kind="ExternalInput")
    bias2_in = nc.dram_tensor("bias2", [P, F2], F32, kind="ExternalInput")
    out_ext = nc.dram_tensor("out_shard", [c.SHARD, F2], F32, kind="ExternalOutput")

    # ---- internal DRAM ----
    def dram(name, shape, dt, shared=False):
        return nc.dram_tensor(name, shape, dt,
                              addr_space="Shared" if shared else "Local")

    FT = [dram("FT0", [c.NPAD, F0], BF16, True),
          dram("FT1", [c.NPAD, F0], BF16, True),
          dram("FT2", [c.NPAD, F2], F32, True)]
    ELR = [dram("ELR0", [c.NPAD, 64], F32),
           dram("ELR1", [c.NPAD, 64], F32),
           dram("ELR2", [c.NPAD, 64], F32)]
    ERR = [dram("ERR0", [c.SHARD, 64], F32),
           dram("ERR1", [c.SHARD, 64], F32),
           dram("ERR2", [c.SHARD, 64], F32)]
    FTS = [dram("FTS0", [c.SHARD, F0], BF16),
           dram("FTS1", [c.SHARD, F0], BF16),
           dram("FTS2", [c.SHARD, F2], F32)]
    ELS = [dram("ELS0", [c.SHARD, H0], F32),
           dram("ELS1", [c.SHARD, H0], F32),
           dram("ELS2", [c.SHARD, H2], F32)]
    ELF = [dram("ELF0", [c.NPAD, H0], F32, True),
           dram("ELF1", [c.NPAD, H0], F32, True),
           dram("ELF2", [c.NPAD, H2], F32, True)]
    HT = [None,
          dram("HT1", [P, c.SHARD], F32),
          dram("HT2", [P, c.SHARD], F32)]

    groups = [list(range(c.NCORES))]

    with tile.TileContext(nc) as tc:
        with (
            tc.tile_pool(name="const", bufs=1) as constp,
            tc.tile_pool(name="tbl", bufs=3) as tblp,
            tc.tile_pool(name="idx", bufs=2) as idxp,
            tc.tile_pool(name="gath", bufs=2) as gathp,
            tc.tile_pool(name="edge", bufs=2) as edgep,
            tc.tile_pool(name="epi", bufs=2) as epip,
            tc.tile_pool(name="psum", bufs=1, space="PSUM") as psump,
        ):
            # ---- constants ----
            ident = constp.tile([P, P], F32, tag="ident")
            make_identity(nc, ident[:])
            iota_f = constp.tile([P, P], F32, tag="iota")
            nc.gpsimd.iota(iota_f[:], pattern=[[1, P]], base=0,
                           channel_multiplier=0,
                           allow_small_or_imprecise_dtypes=True)
            wc_sb = [constp.tile([P, F0 + 2 * H0], F32, tag="wc0", name="wc0s"),
                     constp.tile([P, F0 + 2 * H0], F32, tag="wc1", name="wc1s"),
                     constp.tile([P, F2 + 2 * H2], F32, tag="wc2", name="wc2s")]
            nc.sync.dma_start(wc_sb[0][:], wc0_in[:, :])
            nc.sync.dma_start(wc_sb[1][:], wc1_in[:, :])
            nc.sync.dma_start(wc_sb[2][:], wc2_in[:, :])
            bias_sb = [constp.tile([P, F0], F32, tag="b0", name="b0s"),
                       constp.tile([P, F0], F32, tag="b1", name="b1s"),
                       constp.tile([P, F2], F32, tag="b2", name="b2s")]
            nc.sync.dma_start(bias_sb[0][:], bias0_in[:, :])
            nc.sync.dma_start(bias_sb[1][:], bias1_in[:, :])
            nc.sync.dma_start(bias_sb[2][:], bias2_in[:, :])

            def table_pass(lyr, h_src):
                """ft/el/er for own shard from hT (h_src: DRAM [P, SHARD])."""
                F = F2 if lyr == 2 else F0
                H = H2 if lyr == 2 else H0
                ftdt = F32 if lyr == 2 else BF16
                rep = 64 // H
                for t in range(c.TILES):
                    ht = tblp.tile([P, P], F32, tag="ht_in")
                    nc.sync.dma_start(ht[:], h_src[:, bass.ts(t, P)])
                    ps = psump.tile([P, F + 2 * H], F32, tag="agg0")
                    nc.tensor.matmul(ps[:], lhsT=ht[:], rhs=wc_sb[lyr][:, :],
                                     start=True, stop=True)
                    ft_sb = tblp.tile([P, F], ftdt, tag="ft_sb")
                    nc.vector.tensor_copy(ft_sb[:], ps[:, 0:F])
                    el_sb = tblp.tile([P, H], F32, tag="el_sb")
                    nc.vector.tensor_copy(el_sb[:], ps[:, F:F + H])
                    er_sb = tblp.tile([P, 64], F32, tag="er_sb")
                    src_ap = ps[:, F + H:F + 2 * H].unsqueeze(1).to_broadcast(
                        [P, rep, H])
                    nc.vector.tensor_copy(
                        er_sb[:].rearrange("p (r h) -> p r h", h=H), src_ap)
                    nc.sync.dma_start(FTS[lyr][bass.ts(t, P), :], ft_sb[:])
                    nc.sync.dma_start(ELS[lyr][bass.ts(t, P), :], el_sb[:])
                    nc.sync.dma_start(ERR[lyr][bass.ts(t, P), :], er_sb[:])

            def gather_and_expand(lyr):
                """AllGather ft + el; expand el -> ELR (x rep broadcast)."""
                H = H2 if lyr == 2 else H0
                rep = 64 // H
                nc.gpsimd.collective_compute(
                    "AllGather", mybir.AluOpType.bypass,
                    replica_groups=groups,
                    ins=[FTS[lyr][:, :]], outs=[FT[lyr][:, :]])
                nc.gpsimd.collective_compute(
                    "AllGather", mybir.AluOpType.bypass,
                    replica_groups=groups,
                    ins=[ELS[lyr][:, :]], outs=[ELF[lyr][:, :]])
                # expand: blocks of 1024 rows
                nblk = c.NPAD // (8 * P)
                for blk in range(nblk):
                    el_in = tblp.tile([P, 8 * H], F32, tag="el_in")
                    src = ELF[lyr][bass.ds(blk * 8 * P, 8 * P), :].rearrange(
                        "(p k) h -> p (k h)", p=P)
                    nc.sync.dma_start(el_in[:], src)
                    el_out = tblp.tile([P, 8 * 64], F32, tag="el_out")
                    bcast = el_in[:].rearrange("p (k h) -> p k h", h=H) \
                        .unsqueeze(2).to_broadcast([P, 8, rep, H])
                    nc.vector.tensor_copy(
                        el_out[:].rearrange("p (k r h) -> p k r h", r=rep, h=H),
                        bcast)
                    dst = ELR[lyr][bass.ds(blk * 8 * P, 8 * P), :].rearrange(
                        "(p k) h -> p (k h)", p=P)
                    nc.sync.dma_start(dst, el_out[:])

            def edge_pass(lyr):
                F = F2 if lyr == 2 else F0
                H = H2 if lyr == 2 else H0
                D = D2 if lyr == 2 else D0
                ftdt = F32 if lyr == 2 else BF16
                relu = lyr != 2
                Q = F + H
                for b in range(c.NB):
                    psums = [psump.tile([P, Q], F32, tag=f"agg{ti}", name=f"agg{ti}")
                             for ti in range(c.TB)]
                    for ch in range(c.CHUNKS):
                        bc = b * c.CHUNKS + ch
                        idxf = idxp.tile([P, NIDX // 16], I16, tag="idxf")
                        nc.sync.dma_start(idxf[:], ftidx_in[bc, :, :])
                        idxe = idxp.tile([P, NIDX // 16], I16, tag="idxe")
                        nc.sync.dma_start(idxe[:], erridx_in[bc, :, :])
                        dstl = idxp.tile([P, SLOT], F32, tag="dstl")
                        nc.sync.dma_start(dstl[:], dstl_in[bc, :, :])

                        ftb = gathp.tile([P, SLOT * F], ftdt, tag="ftb")
                        nc.gpsimd.dma_gather(
                            ftb[:].rearrange("p (s f) -> p s f", f=F),
                            FT[lyr][bass.ds(ch * c.CR, c.CR), :],
                            idxf[:], NIDX, NIDX, F, single_packet=False,
                            queue_num=(3 * bc) % 4)
                        elrb = gathp.tile([P, SLOT * 64], F32, tag="elrb")
                        nc.gpsimd.dma_gather(
                            elrb[:].rearrange("p (s f) -> p s f", f=64),
                            ELR[lyr][bass.ds(ch * c.CR, c.CR), :],
                            idxf[:], NIDX, NIDX, 64, single_packet=False,
                            queue_num=(3 * bc + 1) % 4)
                        errb = gathp.tile([P, SLOT * 64], F32, tag="errb")
                        nc.gpsimd.dma_gather(
                            errb[:].rearrange("p (s f) -> p s f", f=64),
                            ERR[lyr][:, :],
                            idxe[:], NIDX, NIDX, 64, single_packet=False,
                            queue_num=(3 * bc + 2) % 4)

                        x = edgep.tile([P, SLOT * H], F32, tag="x")
                        nc.vector.tensor_tensor(
                            x[:].rearrange("p (s h) -> p s h", h=H),
                            elrb[:].rearrange("p (s f) -> p s f", f=64)[:, :, 0:H],
                            errb[:].rearrange("p (s f) -> p s f", f=64)[:, :, 0:H],
                            op=mybir.AluOpType.add)
                        x2 = edgep.tile([P, SLOT * H], F32, tag="x2")
                        nc.vector.scalar_tensor_tensor(
                            x2[:], in0=x[:], scalar=c.NEG, in1=x[:],
                            op0=mybir.AluOpType.mult, op1=mybir.AluOpType.max)
                        exf = edgep.tile([P, SLOT * H], F32, tag="exf")
                        nc.scalar.activation(exf[:], x2[:],
                                             mybir.ActivationFunctionType.Exp)
                        exb = edgep.tile([P, SLOT * H], BF16, tag="exb")
                        nc.vector.tensor_copy(exb[:], exf[:])

                        S = edgep.tile([P, SLOT * P], BF16, tag="S")
                        nc.vector.tensor_tensor(
                            S[:].rearrange("p (s n) -> p s n", n=P),
                            dstl[:].unsqueeze(2).to_broadcast([P, SLOT, P]),
                            iota_f[:].unsqueeze(1).to_broadcast([P, SLOT, P]),
                            op=mybir.AluOpType.is_equal)

                        msgx = edgep.tile([P, SLOT * Q], BF16, tag="msgx")
                        mv = msgx[:].rearrange("p (s q) -> p s q", q=Q)
                        nc.vector.tensor_tensor(
                            mv[:, :, 0:F].rearrange("p s (h d) -> p s h d", d=D),
                            ftb[:].rearrange("p (s h d) -> p s h d", h=H, d=D),
                            exb[:].rearrange("p (s h) -> p s h", h=H)
                                .unsqueeze(3).to_broadcast([P, SLOT, H, D]),
                            op=mybir.AluOpType.mult)
                        nc.vector.tensor_copy(
                            mv[:, :, F:Q],
                            exb[:].rearrange("p (s h) -> p s h", h=H))

                        for ti in range(c.TB):
                            for g in range(G):
                                s = ti * G + g
                                nc.tensor.matmul(
                                    psums[ti][:, :],
                                    lhsT=S[:, bass.ts(s, P)],
                                    rhs=mv[:, s, :],
                                    start=(ch == 0 and g == 0),
                                    stop=(ch == c.CHUNKS - 1 and g == G - 1))
                    # epilogue
                    for ti in range(c.TB):
                        t = b * c.TB + ti
                        rec = epip.tile([P, H], F32, tag="rec")
                        nc.vector.reciprocal(rec[:], psums[ti][:, F:Q])
                        o = epip.tile([P, F], F32, tag="o")
                        nc.vector.tensor_tensor(
                            o[:].rearrange("p (h d) -> p h d", d=D),
                            psums[ti][:, 0:F].rearrange("p (h d) -> p h d", d=D),
                            rec[:].unsqueeze(2).to_broadcast([P, H, D]),
                            op=mybir.AluOpType.mult)
                        o2 = epip.tile([P, F], F32, tag="o2")
                        nc.vector.tensor_tensor(o2[:], o[:], bias_sb[lyr][:, :],
                                                op=mybir.AluOpType.add)
                        if relu:
                            o3 = epip.tile([P, F], F32, tag="o3")
                            nc.scalar.activation(
                                o3[:], o2[:], mybir.ActivationFunctionType.Relu)
                            pst = psump.tile([P, P], F32, tag="ptr")
                            nc.tensor.transpose(pst[:], o3[:], ident[:])
                            htile = epip.tile([P, P], F32, tag="htile")
                            nc.vector.tensor_copy(htile[:], pst[:])
                            nc.sync.dma_start(HT[lyr + 1][:, bass.ts(t, P)],
                                              htile[:])
                        else:
                            nc.sync.dma_start(out_ext[bass.ts(t, P), :], o2[:])

            if bench_compute:
                # timing-only variant: AGs hoisted out, compute looped on-HW
                for lyr in range(3):
                    gather_and_expand(lyr)

                def compute_body(_i):
                    table_pass(0, featT_own)
                    edge_pass(0)
                    table_pass(1, HT[1])
                    edge_pass(1)
                    table_pass(2, HT[2])
                    edge_pass(2)
                with tc.For_i(0, bench_compute, 1) as i:
                    compute_body(i)
            elif bench_ag:
                table_pass(0, featT_own)
                table_pass(1, featT_own)
                table_pass(2, featT_own)
                for _ in range(bench_ag):
                    for lyr in range(3):
                        gather_and_expand(lyr)
            else:
                # ---- layer 0 ----
                table_pass(0, featT_own)
                gather_and_expand(0)
                edge_pass(0)
                # ---- layer 1 ----
                table_pass(1, HT[1])
                gather_and_expand(1)
                edge_pass(1)
                # ---- layer 2 ----
                table_pass(2, HT[2])
                gather_and_expand(2)
                edge_pass(2)

    nc.compile()
    return nc


# ----------------------------------------------------------------------------
# Host entry points
# ----------------------------------------------------------------------------

def make_in_maps(cfg, features, src, dst, weights):
    """weights: dict with W0,al0,ar0,b0,W1,...  Returns list of in_maps."""
    c = cfg
    ftidx, erridx, dstl = prep_edges(c, src, dst)
    wc0 = prep_weights(c, weights["W0"], weights["al0"], weights["ar0"])
    wc1 = prep_weights(c, weights["W1"], weights["al1"], weights["ar1"])
    wc2 = prep_weights(c, weights["W2"], weights["al2"], weights["ar2"])
    b0 = np.tile(np.asarray(weights["b0"], np.float32), (P, 1))
    b1 = np.tile(np.asarray(weights["b1"], np.float32), (P, 1))
    b2 = np.tile(np.asarray(weights["b2"], np.float32), (P, 1))

    featpadT = np.zeros((P, c.NPAD), np.float32)
    featpadT[:, :c.N] = np.asarray(features, np.float32).T

    in_maps = []
    for k in range(c.NCORES):
        in_maps.append({
            "featT_own": np.ascontiguousarray(
                featpadT[:, k * c.SHARD:(k + 1) * c.SHARD]),
            "ftidx": ftidx[k],
            "erridx": erridx[k],
            "dstl": dstl[k],
            "wc0": wc0, "wc1": wc1, "wc2": wc2,
            "bias0": b0, "bias1": b1, "bias2": b2,
        })
    return in_maps


def unshard_output(cfg, results):
    c = cfg
    parts = [results[k]["out_shard"] for k in range(c.NCORES)]
    return np.concatenate(parts, axis=0)[:c.N].astype(np.float32)


def kernel(features, src, dst, W0, al0, ar0, b0, W1, al1, ar1, b1,
           W2, al2, ar2, b2):
    cfg = Cfg(100000, 1600000)
    weights = dict(W0=np.asarray(W0), al0=np.asarray(al0), ar0=np.asarray(ar0),
                   b0=np.asarray(b0), W1=np.asarray(W1), al1=np.asarray(al1),
                   ar1=np.asarray(ar1), b1=np.asarray(b1), W2=np.asarray(W2),
                   al2=np.asarray(al2), ar2=np.asarray(ar2), b2=np.asarray(b2))
    in_maps = make_in_maps(cfg, np.asarray(features), np.asarray(src),
                           np.asarray(dst), weights)
    nc = build_program(cfg)
    res = run_bass_kernel_spmd(nc, in_maps, list(range(cfg.NCORES)))
    return unshard_output(cfg, res.results)
